# revision 28
# baseline (speedup 1.0000x reference)
"""NemotronH Top-k MoE router on 8 Trainium2 NeuronCores.

Strategy
--------
Token-parallel: 16384 tokens are sharded 2048-per-core across 8 cores;
the router weight [256, 4096] and bias are replicated. No collectives.

Matmul (the compute-dominant part): logits = hidden @ weight.T needs
full fp32 accuracy (top-k selection margins are ~1e-7), but the PE's
native fp32 matmul is 4x slower than fp16. So hidden and weight are
split host-side into fp16 hi/lo planes (x = h + l/2048, 22 mantissa
bits) and the product is computed in three fp16 passes accumulated in
fp32 PSUM:  logits = h@u + (h@v + l@u)/2048, dropping the O(2^-22) l@v
term. Subnormal fp16 values are flushed to zero host-side so PE FTZ
behavior cannot skew the split.

Layout: hidden is the STATIONARY matmul operand ([h, t] chunks), the
weight streams as the moving operand, so logits land [token, expert]
in PSUM directly -- no PE transposes. The moving operand packs
[wh | wl] side by side (N=512): one instruction accumulates both the
main term (cols 0:256) and the h@v correction (cols 256:512); the l@u
pass then accumulates into the same correction half at N=256.

Routing per 128-token subtile (tokens on partitions, experts on free
axis), pipelined against the next subtile's matmuls: ACT sigmoid from
PSUM, then DVE/Pool ops: group top-2 via reduce_max + match_replace,
group top-4 via max8 threshold, top-8 via max8 + max_index, per-slot
score extraction via is_equal + accumulate, then normalize.
"""
import sys
sys.path.insert(0, "/opt/trn_rl_repo")

import numpy as np

from concourse import bacc, tile, mybir
from concourse.bass_utils import run_bass_kernel_spmd

F32 = mybir.dt.float32
F16 = mybir.dt.float16
U16 = mybir.dt.uint16
I32 = mybir.dt.int32
AF = mybir.ActivationFunctionType
ALU = mybir.AluOpType

T_TOTAL = 16384
H = 4096
E = 256
G, GS = 8, 32
TOP_K = 8
N_CORES = 8
T_CORE = T_TOTAL // N_CORES      # 2048
TT = 512                         # tokens per DMA tile
N_TILES = T_CORE // TT           # 4
NSUB = TT // 128                 # 4 routing subtiles per DMA tile
KC = H // 128                    # 32 k-chunks
KH = KC // 2                     # 16 per k-half
S = 2048.0                       # lo-plane scale (2^11)
ROUTED_SCALING = 2.5


def build_program(reps=1):
    nc = bacc.Bacc("TRN2", target_bir_lowering=False)
    hst_h = nc.dram_tensor("hst_h", [H, T_CORE], F16, kind="ExternalInput")
    hst_l = nc.dram_tensor("hst_l", [H, T_CORE], F16, kind="ExternalInput")
    wcomb = nc.dram_tensor("wcomb", [H, 2 * E], F16, kind="ExternalInput")
    bias_d = nc.dram_tensor("bias_bc", [128, E], F32, kind="ExternalInput")
    iota_d = nc.dram_tensor("iota_bc", [128, E], F32, kind="ExternalInput")
    idx_out = nc.dram_tensor("idx_out", [T_CORE, TOP_K], I32, kind="ExternalOutput")
    w_out = nc.dram_tensor("w_out", [T_CORE, TOP_K], F32, kind="ExternalOutput")

    with tile.TileContext(nc) as tc:
        with (
            tc.tile_pool(name="const", bufs=1) as cpool,
            tc.tile_pool(name="hs", bufs=2) as hspool,
            tc.tile_pool(name="rt", bufs=2) as rt,
            tc.tile_pool(name="outp", bufs=2) as outp,
            tc.tile_pool(name="ps", bufs=3, space="PSUM") as ps,
            tc.tile_pool(name="psw", bufs=1, space="PSUM") as psw,
        ):
            if reps == 1:
                # pre-warm the PE clock (HAM) with dummy matmuls on zeroed
                # tiles; they run during the startup DMA wait, so the real
                # stream starts at full clock
                warm_s = cpool.tile([128, 128], F16)
                warm_m = cpool.tile([128, 512], F16)
                nc.gpsimd.memset(warm_s[:], 0)
                nc.gpsimd.memset(warm_m[:], 0)
                warm_ps = psw.tile([128, 512], F32, tag="warmps")
                for i in range(8):
                    nc.tensor.matmul(warm_ps[:], warm_s[:], warm_m[:],
                                     start=(i == 0), stop=(i == 7))
            # const DMAs, split + interleaved with the first hidden tile so
            # the first matmul's data arrives early
            wc_t = cpool.tile([128, KC, 2 * E], F16)
            bias_t = cpool.tile([128, E], F32)
            iota_t = cpool.tile([128, E], F32)
            wsrc = wcomb.rearrange("(c p) n -> p c n", p=128)
            QC = KC // 8
            interleave = reps == 1

            def dma_consts(rng, small=False):
                for q in rng:
                    nc.sync.dma_start(wc_t[:, q * QC:(q + 1) * QC, :],
                                      wsrc[:, q * QC:(q + 1) * QC, :])
                if small:
                    nc.sync.dma_start(bias_t[:], bias_d[:])
                    nc.sync.dma_start(iota_t[:], iota_d[:])

            if interleave:
                # only the first weight eighth before the first hidden chunk;
                # the rest interleaves with tile 0's loads inside body()
                dma_consts([0])
            else:
                dma_consts(range(8), small=True)

            def body():
                for it in range(N_TILES):
                    t0 = it * TT
                    # ---- load hidden planes, split in k-halves for pipelining
                    hh = [hspool.tile([128, KH, TT], F16, tag=f"hh{k}", name=f"hh{k}")
                          for k in range(2)]
                    hl = [hspool.tile([128, KH, TT], F16, tag=f"hl{k}", name=f"hl{k}")
                          for k in range(2)]
                    for k in range(2):
                        src_h = hst_h[k * KH * 128:(k + 1) * KH * 128, t0:t0 + TT]
                        src_l = hst_l[k * KH * 128:(k + 1) * KH * 128, t0:t0 + TT]
                        if it == 0:
                            # split the hi-plane loads so the earliest
                            # k-chunks land ASAP; the reps==1 path splits
                            # finer and rides the weight eighths in between,
                            # the loop path keeps the coarser verified shape
                            srcr = src_h.rearrange("(c p) t -> p c t", p=128)
                            nq = 4 if interleave else 2
                            for q in range(nq):
                                qs = slice(q * KH // nq, (q + 1) * KH // nq)
                                nc.sync.dma_start(hh[k][:, qs, :], srcr[:, qs, :])
                                if interleave:
                                    wi = 4 * k + q + 1
                                    dma_consts([wi] if wi < 8 else [],
                                               small=(wi == 7))
                        else:
                            nc.sync.dma_start(hh[k][:], src_h.rearrange("(c p) t -> p c t", p=128))
                        nc.sync.dma_start(hl[k][:], src_l.rearrange("(c p) t -> p c t", p=128))

                    iouts = outp.tile([128, NSUB, TOP_K], I32, tag="iouts")
                    wouts = outp.tile([128, NSUB, TOP_K], F32, tag="wouts")
                    for sub in range(NSUB):
                        ts = slice(sub * 128, (sub + 1) * 128)
                        # ---- matmuls: [t, e] logits; hidden stationary
                        # ps1[:, 0:256] = sum_c hh_c @ wh_c   (main)
                        # ps1[:, 256:512] = sum_c hh_c @ wl_c + sum_c hl_c @ wh_c
                        ps1 = ps.tile([128, 2 * E], F32, tag="ps1")
                        for c in range(KC):
                            k, ci = divmod(c, KH)
                            nc.tensor.matmul(
                                ps1[:], hh[k][:, ci, ts], wc_t[:, c, :],
                                start=(c == 0), stop=False)
                        for c in range(KC):
                            k, ci = divmod(c, KH)
                            nc.tensor.matmul(
                                ps1[:, E:2 * E], hl[k][:, ci, ts], wc_t[:, c, 0:E],
                                start=False, stop=(c == KC - 1))

                        # ---- combine + sigmoid (one PSUM read per op — a
                        # single STT reading both PSUM halves fails walrus
                        # codegen; keep ACT on Sigmoid only so its function
                        # table never reloads mid-kernel)
                        corr_s = rt.tile([128, E], F32, tag="corr_s")
                        nc.vector.tensor_scalar_mul(corr_s[:], ps1[:, E:2 * E],
                                                    1.0 / S)
                        scomb = rt.tile([128, E], F32, tag="scomb")
                        nc.vector.scalar_tensor_tensor(
                            scomb[:], corr_s[:], 1.0, ps1[:, 0:E],
                            ALU.mult, ALU.add)
                        scores = rt.tile([128, E], F32, tag="scores")
                        nc.scalar.activation(scores[:], scomb[:], AF.Sigmoid)

                        s4c = rt.tile([128, E], F32, tag="s4c")
                        nc.vector.tensor_tensor(s4c[:], scores[:], bias_t[:], ALU.add)

                        # ---- group top-2 -> group scores -> top-4 groups
                        m1 = rt.tile([128, G], F32, tag="m1")
                        nc.vector.reduce_max(
                            m1[:], s4c[:].rearrange("p (g s) -> p g s", g=G),
                            axis=mybir.AxisListType.X)
                        s4cr = rt.tile([128, E], F32, tag="s4cr")
                        nc.vector.match_replace(s4cr[:], m1[:], s4c[:], -1e30)
                        m2 = rt.tile([128, G], F32, tag="m2")
                        nc.vector.reduce_max(
                            m2[:], s4cr[:].rearrange("p (g s) -> p g s", g=G),
                            axis=mybir.AxisListType.X)
                        gsc = rt.tile([128, G], F32, tag="gsc")
                        nc.vector.tensor_tensor(gsc[:], m1[:], m2[:], ALU.add)

                        gsorted = rt.tile([128, 8], F32, tag="gsorted")
                        nc.vector.max(gsorted[:], gsc[:])
                        gmask = rt.tile([128, G], F32, tag="gmask")
                        nc.vector.tensor_scalar(
                            gmask[:], gsc[:], gsorted[:, 3:4], None, ALU.is_ge)

                        masked = rt.tile([128, E], F32, tag="masked")
                        nc.vector.tensor_tensor(
                            masked[:].rearrange("p (g s) -> p g s", g=G),
                            s4c[:].rearrange("p (g s) -> p g s", g=G),
                            gmask[:].unsqueeze(-1).broadcast_to([128, G, GS]),
                            ALU.mult)

                        # ---- top-8 values + indices
                        vals = rt.tile([128, 8], F32, tag="vals")
                        nc.vector.max(vals[:], masked[:])
                        idx16 = rt.tile([128, 8], U16, tag="idx16")
                        nc.vector.max_index(idx16[:], vals[:], masked[:])

                        # per-slot gather scores[idx[k]]: match idx against an
                        # iota row (unique values -> tie-safe), accumulate
                        idxf = rt.tile([128, 8], F32, tag="idxf")
                        nc.vector.tensor_copy(idxf[:], idx16[:])
                        w8 = rt.tile([128, 8], F32, tag="w8")
                        scratch = rt.tile([128, E], F32, tag="scratch")
                        for k in range(TOP_K):
                            nc.vector.scalar_tensor_tensor(
                                scratch[:], iota_t[:], idxf[:, k:k + 1], scores[:],
                                ALU.is_equal, ALU.mult,
                                accum_out=w8[:, k:k + 1])

                        denom = rt.tile([128, 1], F32, tag="denom")
                        nc.vector.reduce_sum(denom[:], w8[:], axis=mybir.AxisListType.X)
                        # rec = ROUTED_SCALING / (denom + 1e-20) in two ops:
                        # fold eps and 1/2.5 into one tensor_scalar, then recip
                        rec = rt.tile([128, 1], F32, tag="rec")
                        nc.vector.tensor_scalar(
                            denom[:], denom[:], 1.0 / ROUTED_SCALING,
                            1e-20 / ROUTED_SCALING, ALU.mult, ALU.add)
                        nc.vector.reciprocal(rec[:], denom[:])

                        nc.vector.tensor_scalar(
                            wouts[:, sub, :], w8[:], rec[:, 0:1], None, ALU.mult)
                        nc.vector.tensor_copy(iouts[:, sub, :], idx16[:])

                    nc.sync.dma_start(
                        idx_out[t0:t0 + TT, :].rearrange("(s p) k -> p s k", p=128),
                        iouts[:])
                    nc.sync.dma_start(
                        w_out[t0:t0 + TT, :].rearrange("(s p) k -> p s k", p=128),
                        wouts[:])

            if reps == 1:
                body()
            elif reps % 4 == 0:
                # 4 bodies per hardware-loop iteration: the all-engine
                # barrier runs once per 4 reps, and adjacent bodies overlap
                # (DMA prefetch of body n+1 under body n's routing tail)
                with tc.For_i(0, reps // 4, 1):
                    for _ in range(4):
                        body()
            else:
                with tc.For_i(0, reps, 1):
                    body()
    nc.compile()
    return nc


_PROGRAM_CACHE = {}


def _get_program(reps=1):
    if reps not in _PROGRAM_CACHE:
        _PROGRAM_CACHE[reps] = build_program(reps)
    return _PROGRAM_CACHE[reps]


_F16_MIN_NORMAL = 2.0 ** -14


def _split_f16(x):
    """x (f32) -> (h, l) fp16 planes with x ~= h + l/S; subnormals zeroed."""
    h = x.astype(np.float16)
    h32 = h.astype(np.float32)
    h = np.where(np.abs(h32) < _F16_MIN_NORMAL, np.float16(0), h)
    h32 = h.astype(np.float32)
    l = ((x - h32) * np.float32(S)).astype(np.float16)
    l32 = l.astype(np.float32)
    l = np.where(np.abs(l32) < _F16_MIN_NORMAL, np.float16(0), l)
    return h, l


def _prepare_inputs(hidden_states, weight, e_score_correction_bias):
    hs = np.asarray(hidden_states, dtype=np.float32)
    w = np.asarray(weight, dtype=np.float32)
    b = np.asarray(e_score_correction_bias, dtype=np.float32)

    wh, wl = _split_f16(w)
    wcomb = np.empty((H, 2 * E), dtype=np.float16)
    wcomb[:, 0:E] = wh.T
    wcomb[:, E:2 * E] = wl.T
    bias_bc = np.ascontiguousarray(np.broadcast_to(b, (128, E)))
    iota_bc = np.ascontiguousarray(
        np.broadcast_to(np.arange(E, dtype=np.float32), (128, E)))

    in_maps = []
    for c in range(N_CORES):
        sl = hs[c * T_CORE:(c + 1) * T_CORE]  # [T_CORE, H]
        h, l = _split_f16(sl)
        in_maps.append({
            "hst_h": np.ascontiguousarray(h.T),
            "hst_l": np.ascontiguousarray(l.T),
            "wcomb": wcomb,
            "bias_bc": bias_bc,
            "iota_bc": iota_bc,
        })
    return in_maps


def kernel(hidden_states, weight, e_score_correction_bias):
    in_maps = _prepare_inputs(hidden_states, weight, e_score_correction_bias)
    nc = _get_program(1)
    res = run_bass_kernel_spmd(nc, in_maps, list(range(N_CORES)))
    idx = np.concatenate([r["idx_out"] for r in res.results], axis=0)
    w = np.concatenate([r["w_out"] for r in res.results], axis=0)
    return idx.astype(np.int32), w.astype(np.float32)
